# revision 15
# baseline (speedup 1.0000x reference)
"""EuclideanCodebook (VQ) kernel for 8 TRN2 NeuronCores.

Problem: x [8, 4096, 256] f32, embedding [2048, 256] f32.
  xf = x.reshape(-1, 256); dist = |xf|^2 + |e|^2 - 2 xf @ e.T
  codes = argmin(dist, -1) (int32); quantized = embedding[codes]
  returns (quantized, xf, codes)

Sharding: data-parallel on tokens, 4096 per core; embedding replicated.
The host also supplies transposed copies of the shard / embedding (pure
data movement) so the device needs no on-chip transposes.

Numerics: the TensorEngine's fast paths are reduced-precision.  We use
an explicit Dekker-style split x = xh + xl, e = eh + el (bf16 hi/lo)
and compute x.e = xh.eh + xh.el + xl.eh with three bf16 matmuls
accumulated in fp32 PSUM.  On this problem the worst-case argmin margin
(2.8e-4) is ~30x the residual error, and the -|e|^2/2 bias is applied
EXACTLY (fp32) by preloading PSUM via the scalar engine before the
accumulation group.  argmax(x.e - |e|^2/2) == argmin(dist).

Per-tile main loop (32 tiles of 128 tokens):
 - DMA xT tile [128d x 128t x 2]; ACT casts hi, DVE computes lo
 - per 512-code chunk: ACT preloads PSUM with -|e|^2/2; 6 bf16
   matmuls accumulate the three split passes over two 128-d halves;
   chunk copied to SBUF scores (split between ACT and DVE)
 - DVE max8 + find_index8 -> code per token (argmin incl. first-index
   tie-break, matching jnp.argmin)
 - gpsimd indirect-DMA gathers embedding[code] -> quantized rows
 - q rows + codes column DMA out

Token <-> (partition, tile) mapping: local token n = t*128 + p.
"""

import numpy as np
from contextlib import ExitStack

P = 128
D = 256
K = 2048
N_CORES = 8
TOK_PER_CORE = 4096
NCHUNK = 4
CW = 512
KCH = 2

MM_MODE = "bf16x3"  # "f32" | "f32r" | "bf16x3"
DVE_OUTS = 2  # how many of the 4 chunk copy-outs run on DVE (rest ACT)


def build(n_tok=TOK_PER_CORE, mm_mode=MM_MODE, dve_outs=DVE_OUTS):
    import concourse.bass as bass
    import concourse.tile as tile
    from concourse import bacc, mybir
    from concourse.bass import ts

    f32 = mybir.dt.float32
    bf16 = mybir.dt.bfloat16
    u32 = mybir.dt.uint32

    TT = n_tok // P

    nc = bacc.Bacc(
        "TRN2", target_bir_lowering=False, debug=False, num_devices=N_CORES
    )

    xt_d = nc.dram_tensor("x_t", [D, n_tok], f32, kind="ExternalInput").ap()
    e_d = nc.dram_tensor("emb", [K, D], f32, kind="ExternalInput").ap()
    et_d = nc.dram_tensor("emb_t", [D, K], f32, kind="ExternalInput").ap()
    q_d = nc.dram_tensor("q", [n_tok, D], f32, kind="ExternalOutput").ap()
    c_d = nc.dram_tensor("codes", [TT, P], u32, kind="ExternalOutput").ap()

    with tile.TileContext(nc) as tc, ExitStack() as ctx:
        const = ctx.enter_context(tc.tile_pool(name="const", bufs=1))
        xTpool = ctx.enter_context(tc.tile_pool(name="xTpool", bufs=3))
        spool = ctx.enter_context(tc.tile_pool(name="spool", bufs=3))
        qpool = ctx.enter_context(tc.tile_pool(name="qpool", bufs=3))
        ipool = ctx.enter_context(tc.tile_pool(name="ipool", bufs=4))
        junk = ctx.enter_context(tc.tile_pool(name="junk", bufs=2))
        psum_b2 = ctx.enter_context(tc.tile_pool(name="psum_b2", bufs=2, space="PSUM"))
        psum_mm = ctx.enter_context(tc.tile_pool(name="psum_mm", bufs=6, space="PSUM"))

        ones_col = const.tile([P, 1], f32, tag="ones_col")
        nc.vector.memset(ones_col[:], 1.0)
        ones_row = const.tile([1, P], f32, tag="ones_row")
        nc.vector.memset(ones_row[:], 1.0)

        if mm_mode == "bf16x3":
            eh = [
                [
                    const.tile([P, CW], bf16, tag=f"eh{c}_{n}", name=f"eh{c}_{n}")
                    for n in range(NCHUNK)
                ]
                for c in range(KCH)
            ]
            el = [
                [
                    const.tile([P, CW], bf16, tag=f"el{c}_{n}", name=f"el{c}_{n}")
                    for n in range(NCHUNK)
                ]
                for c in range(KCH)
            ]
        else:
            mmdt = f32 if mm_mode == "f32" else mybir.dt.float32r
            eTr = [
                [
                    const.tile([P, CW], mmdt, tag=f"eTr{c}_{n}", name=f"eTr{c}_{n}")
                    for n in range(NCHUNK)
                ]
                for c in range(KCH)
            ]
        nhb2_row = const.tile([1, K], f32, tag="nhb2_row")
        nhb2 = [
            const.tile([P, CW], f32, tag=f"nhb2_{n}", name=f"nhb2_{n}")
            for n in range(NCHUNK)
        ]

        # ---- stage A: load e.T chunks, split hi/lo, compute -|e|^2/2 ----
        for nch in range(NCHUNK):
            ef = []
            for c in range(KCH):
                etile = junk.tile([P, CW], f32, tag="etile", name=f"etile{c}_{nch}")
                nc.sync.dma_start(etile[:], et_d[ts(c, P), ts(nch, CW)])
                ef.append(etile)
                if mm_mode == "bf16x3":
                    nc.scalar.copy(eh[c][nch][:], etile[:])
                    nc.vector.tensor_sub(el[c][nch][:], etile[:], eh[c][nch][:])
                elif mm_mode == "f32r":
                    nc.scalar.copy(eTr[c][nch][:], etile[:])
                else:
                    nc.scalar.copy(eTr[c][nch][:], etile[:])
            bp = psum_b2.tile([1, CW], f32, tag="b2")
            for c in range(KCH):
                sq = junk.tile([P, CW], f32, tag="sq")
                nc.vector.tensor_mul(sq[:], ef[c][:], ef[c][:])
                nc.tensor.matmul(
                    bp[:],
                    lhsT=ones_col[:],
                    rhs=sq[:],
                    start=(c == 0),
                    stop=(c == KCH - 1),
                )
            nc.scalar.mul(nhb2_row[0:1, ts(nch, CW)], bp[:], -0.5)
            bb = psum_mm.tile([P, CW], f32, tag="mm")
            nc.tensor.matmul(
                bb[:],
                lhsT=ones_row[:],
                rhs=nhb2_row[0:1, ts(nch, CW)],
                start=True,
                stop=True,
            )
            nc.scalar.copy(nhb2[nch][:], bb[:])

        # ---- main loop ----
        for t in range(TT):
            xTf = xTpool.tile([P, D], f32, tag="xTf")
            for c in range(KCH):
                nc.sync.dma_start(xTf[:, ts(c, P)], xt_d[ts(c, P), ts(t, P)])
            if mm_mode == "bf16x3":
                xh = xTpool.tile([P, D], bf16, tag="xh")
                nc.scalar.copy(xh[:], xTf[:])
                xl = xTpool.tile([P, D], bf16, tag="xl")
                nc.vector.tensor_sub(xl[:], xTf[:], xh[:])
            elif mm_mode == "f32r":
                xT = xTpool.tile([P, D], mmdt, tag="xTr")
                nc.scalar.copy(xT[:], xTf[:])
            else:
                xT = xTf

            scores = spool.tile([P, K], f32, tag="scores")
            for nch in range(NCHUNK):
                mm = psum_mm.tile([P, CW], f32, tag="mm")
                nc.scalar.copy(mm[:], nhb2[nch][:])  # exact bias preload
                if mm_mode == "bf16x3":
                    passes = [(xh, eh), (xh, el), (xl, eh)]
                    for pi, (xa, ea) in enumerate(passes):
                        for c in range(KCH):
                            nc.tensor.matmul(
                                mm[:],
                                lhsT=xa[:, ts(c, P)],
                                rhs=ea[c][nch][:],
                                start=False,
                                stop=(pi == 2 and c == KCH - 1),
                                skip_group_check=True,
                            )
                else:
                    for c in range(KCH):
                        nc.tensor.matmul(
                            mm[:],
                            lhsT=xT[:, ts(c, P)],
                            rhs=eTr[c][nch][:],
                            start=False,
                            stop=(c == KCH - 1),
                            skip_group_check=True,
                        )
                if nch < dve_outs:
                    nc.vector.tensor_copy(scores[:, ts(nch, CW)], mm[:])
                else:
                    nc.scalar.copy(scores[:, ts(nch, CW)], mm[:])

            v8 = ipool.tile([P, 8], f32, tag="v8")
            nc.vector.max(v8[:], scores[:])
            idx8 = ipool.tile([P, 8], u32, tag="idx8")
            nc.vector.max_index(idx8[:], v8[:], scores[:])

            qt = qpool.tile([P, D], f32, tag="qt")
            nc.gpsimd.indirect_dma_start(
                out=qt[:],
                out_offset=None,
                in_=e_d[:, :],
                in_offset=bass.IndirectOffsetOnAxis(ap=idx8[:, 0:1], axis=0),
            )
            nc.sync.dma_start(q_d[ts(t, P), :], qt[:])
            nc.sync.dma_start(c_d[t : t + 1, :], idx8[:, 0:1])

    nc.compile()
    return nc


def kernel(x, embedding):
    from concourse.bass_utils import run_bass_kernel_spmd

    x = np.ascontiguousarray(np.asarray(x, dtype=np.float32))
    e = np.ascontiguousarray(np.asarray(embedding, dtype=np.float32))
    xf = x.reshape(-1, D)
    n_total = xf.shape[0]
    assert n_total == N_CORES * TOK_PER_CORE and e.shape == (K, D)

    e_t = np.ascontiguousarray(e.T)
    nc = build()
    in_maps = []
    for i in range(N_CORES):
        shard = xf[i * TOK_PER_CORE : (i + 1) * TOK_PER_CORE]
        in_maps.append(
            {"x_t": np.ascontiguousarray(shard.T), "emb": e, "emb_t": e_t}
        )
    res = run_bass_kernel_spmd(nc, in_maps, list(range(N_CORES))).results

    q = np.concatenate([res[i]["q"] for i in range(N_CORES)], axis=0)
    codes = np.concatenate(
        [res[i]["codes"].reshape(-1) for i in range(N_CORES)], axis=0
    ).astype(np.int32)
    return q, xf, codes


# revision 22
# speedup vs baseline: 1.2562x; 1.2562x over previous
"""EuclideanCodebook (VQ) kernel for 8 TRN2 NeuronCores.

Problem: x [8, 4096, 256] f32, embedding [2048, 256] f32.
  xf = x.reshape(-1, 256); dist = |xf|^2 + |e|^2 - 2 xf @ e.T
  codes = argmin(dist, -1) (int32); quantized = embedding[codes]
  returns (quantized, xf, codes)

Sharding: data-parallel on tokens, 4096 per core; embedding replicated.

Numerics: the TensorEngine's fast paths are reduced precision, so the
x.e matmul uses an explicit Dekker-style split x = xh + xl, e = eh + el
(bf16 hi + bf16 residual) and computes xh.eh + xh.el + xl.eh with three
bf16 matmuls accumulated in fp32 PSUM; the -|e|^2/2 bias is applied
EXACTLY in fp32 by preloading PSUM via the scalar engine before each
accumulation group. On this problem the worst-case argmin margin
(2.8e-4) is ~30x the residual split error. argmax(x.e - |e|^2/2) ==
argmin(dist), and ties resolve to the first index, matching jnp.argmin.

The host ships the shard/embedding in transposed layout and as bf16
hi/lo pairs (the same input snapping the TensorEngine would perform on
chip); |e|^2 itself is computed on device from the f32 embedding.

Per-core device pipeline:
  stage A: DMA eh/el chunk tiles; DVE squares f32 e.T chunks + PE
  ones-matmul -> -|e_k|^2/2 row; PE broadcast matmul -> per-chunk
  [128,512] f32 bias tiles.
  main loop (32 tiles of 128 tokens):
   - DMA xh/xl tile [128d x 128t x 2 halves]
   - per 512-code chunk: ACT preloads PSUM with the f32 bias; 6 bf16
     matmuls accumulate the three split passes; ACT copies the chunk
     to SBUF scores.
   - DVE max8 + find_index8 (DVE runs ONLY these two ops back-to-back;
     interleaving any other DVE op halves their throughput)
   - gpsimd indirect-DMA gathers embedding[code] -> quantized rows
   - q rows (sync) + codes column (gpsimd) DMA out

Token <-> (partition, tile) mapping: local token n = t*128 + p.
"""

import numpy as np
from contextlib import ExitStack

P = 128
D = 256
K = 2048
N_CORES = 8
TOK_PER_CORE = 4096
NCHUNK = 4
CW = 512
KCH = 2


def build(n_tok=TOK_PER_CORE):
    import concourse.bass as bass
    import concourse.tile as tile
    from concourse import bacc, mybir
    from concourse.bass import ts

    f32 = mybir.dt.float32
    bf16 = mybir.dt.bfloat16
    u32 = mybir.dt.uint32

    TT = n_tok // P

    nc = bacc.Bacc(
        "TRN2", target_bir_lowering=False, debug=False, num_devices=N_CORES
    )

    xh_d = nc.dram_tensor("xh_t", [D, n_tok], bf16, kind="ExternalInput").ap()
    xl_d = nc.dram_tensor("xl_t", [D, n_tok], bf16, kind="ExternalInput").ap()
    e_d = nc.dram_tensor("emb", [K, D], f32, kind="ExternalInput").ap()
    et_d = nc.dram_tensor("emb_t", [D, K], f32, kind="ExternalInput").ap()
    eh_d = nc.dram_tensor("eh_t", [D, K], bf16, kind="ExternalInput").ap()
    el_d = nc.dram_tensor("el_t", [D, K], bf16, kind="ExternalInput").ap()
    q_d = nc.dram_tensor("q", [n_tok, D], f32, kind="ExternalOutput").ap()
    c_d = nc.dram_tensor("codes", [TT, P], u32, kind="ExternalOutput").ap()

    with tile.TileContext(nc) as tc, ExitStack() as ctx:
        const = ctx.enter_context(tc.tile_pool(name="const", bufs=1))
        xTpool = ctx.enter_context(tc.tile_pool(name="xTpool", bufs=4))
        spool = ctx.enter_context(tc.tile_pool(name="spool", bufs=3))
        qpool = ctx.enter_context(tc.tile_pool(name="qpool", bufs=3))
        ipool = ctx.enter_context(tc.tile_pool(name="ipool", bufs=4))
        junk = ctx.enter_context(tc.tile_pool(name="junk", bufs=2))
        epool = ctx.enter_context(tc.tile_pool(name="epool", bufs=4))
        sqpool = ctx.enter_context(tc.tile_pool(name="sqpool", bufs=4))
        psum_b2 = ctx.enter_context(tc.tile_pool(name="psum_b2", bufs=2, space="PSUM"))
        psum_mm = ctx.enter_context(tc.tile_pool(name="psum_mm", bufs=6, space="PSUM"))

        ones_col = const.tile([P, 1], f32, tag="ones_col")
        nc.vector.memset(ones_col[:], 1.0)
        ones_row = const.tile([1, P], f32, tag="ones_row")
        nc.vector.memset(ones_row[:], 1.0)

        eh = [
            [
                const.tile([P, CW], bf16, tag=f"eh{c}_{n}", name=f"eh{c}_{n}")
                for n in range(NCHUNK)
            ]
            for c in range(KCH)
        ]
        el = [
            [
                const.tile([P, CW], bf16, tag=f"el{c}_{n}", name=f"el{c}_{n}")
                for n in range(NCHUNK)
            ]
            for c in range(KCH)
        ]
        # per-chunk bias rows: a single [1, K] tile would serialize the four
        # mul -> broadcast chains through whole-tile WAR tracking
        nhb2_row = [
            const.tile([1, CW], f32, tag=f"nhb2_row{n}", name=f"nhb2_row{n}")
            for n in range(NCHUNK)
        ]
        zrow = const.tile([1, CW], f32, tag="zrow")
        nc.vector.memset(zrow[:], 0.0)
        nhb2 = [
            const.tile([P, CW], f32, tag=f"nhb2_{n}", name=f"nhb2_{n}")
            for n in range(NCHUNK)
        ]

        # Warm TWO spare PSUM slots with a start=True matmul each, first
        # thing (independent of stage A): the main loop accumulates with
        # start=False onto ACT-preloaded banks, which only adds (rather
        # than replaces) when the bank's has_written bits are already set
        # by a prior TensorEngine write.  The four stage-A broadcast
        # matmuls below warm the other four slots.
        for w in range(2):
            wb = psum_mm.tile([P, CW], f32, tag="mm", name=f"warm{w}")
            nc.tensor.matmul(
                wb[:], lhsT=ones_row[:], rhs=zrow[:], start=True, stop=True
            )
            wj = junk.tile([P, CW], f32, tag="wj", name=f"warmj{w}")
            nc.scalar.copy(wj[:], wb[:])

        # ---- stage A: load split e.T chunks, compute -|e|^2/2 ----
        # Spread stage-A loads over all three DMA queues so the sync queue
        # can start feeding the main loop's x tiles early.
        for nch in range(NCHUNK):
            for c in range(KCH):
                nc.sync.dma_start(eh[c][nch][:], eh_d[ts(c, P), ts(nch, CW)])
                nc.gpsimd.dma_start(el[c][nch][:], el_d[ts(c, P), ts(nch, CW)])
            bp = psum_b2.tile([1, CW], f32, tag="b2")
            for c in range(KCH):
                etile = epool.tile([P, CW], f32, tag="etile")
                nc.scalar.dma_start(etile[:], et_d[ts(c, P), ts(nch, CW)])
                sq = sqpool.tile([P, CW], f32, tag="sq")
                nc.scalar.activation(
                    sq[:], etile[:], mybir.ActivationFunctionType.Square
                )
                nc.tensor.matmul(
                    bp[:],
                    lhsT=ones_col[:],
                    rhs=sq[:],
                    start=(c == 0),
                    stop=(c == KCH - 1),
                )
            nc.scalar.mul(nhb2_row[nch][:], bp[:], -0.5)
            bb = psum_mm.tile([P, CW], f32, tag="mm")
            nc.tensor.matmul(
                bb[:],
                lhsT=ones_row[:],
                rhs=nhb2_row[nch][:],
                start=True,
                stop=True,
            )
            nc.scalar.copy(nhb2[nch][:], bb[:])

        # ---- main loop ----
        for t in range(TT):
            xh = xTpool.tile([P, D], bf16, tag="xh")
            xl = xTpool.tile([P, D], bf16, tag="xl")
            for c in range(KCH):
                nc.sync.dma_start(xh[:, ts(c, P)], xh_d[ts(c, P), ts(t, P)])
                nc.sync.dma_start(xl[:, ts(c, P)], xl_d[ts(c, P), ts(t, P)])

            scores = spool.tile([P, K], f32, tag="scores")
            for nch in range(NCHUNK):
                mm = psum_mm.tile([P, CW], f32, tag="mm")
                nc.scalar.copy(mm[:], nhb2[nch][:])  # exact f32 bias preload
                passes = [(xh, eh), (xh, el), (xl, eh)]
                for pi, (xa, ea) in enumerate(passes):
                    for c in range(KCH):
                        nc.tensor.matmul(
                            mm[:],
                            lhsT=xa[:, ts(c, P)],
                            rhs=ea[c][nch][:],
                            start=False,
                            stop=(pi == 2 and c == KCH - 1),
                            skip_group_check=True,
                        )
                nc.scalar.copy(scores[:, ts(nch, CW)], mm[:])

            v8 = ipool.tile([P, 8], f32, tag="v8")
            nc.vector.max(v8[:], scores[:])
            idx8 = ipool.tile([P, 8], u32, tag="idx8")
            nc.vector.max_index(idx8[:], v8[:], scores[:])

            qt = qpool.tile([P, D], f32, tag="qt")
            nc.gpsimd.indirect_dma_start(
                out=qt[:],
                out_offset=None,
                in_=e_d[:, :],
                in_offset=bass.IndirectOffsetOnAxis(ap=idx8[:, 0:1], axis=0),
            )
            nc.sync.dma_start(q_d[ts(t, P), :], qt[:])
            nc.gpsimd.dma_start(c_d[t : t + 1, :], idx8[:, 0:1])

    nc.compile()
    return nc


def _split_bf16(a):
    import ml_dtypes

    hi = a.astype(ml_dtypes.bfloat16)
    lo = (a - hi.astype(np.float32)).astype(ml_dtypes.bfloat16)
    return hi, lo


def make_in_maps(xf, e):
    e_t = np.ascontiguousarray(e.T)
    eh_t, el_t = _split_bf16(e_t)
    eh_t, el_t = np.ascontiguousarray(eh_t), np.ascontiguousarray(el_t)
    in_maps = []
    for i in range(N_CORES):
        shard_t = np.ascontiguousarray(
            xf[i * TOK_PER_CORE : (i + 1) * TOK_PER_CORE].T
        )
        xh_t, xl_t = _split_bf16(shard_t)
        in_maps.append(
            {
                "xh_t": np.ascontiguousarray(xh_t),
                "xl_t": np.ascontiguousarray(xl_t),
                "emb": e,
                "emb_t": e_t,
                "eh_t": eh_t,
                "el_t": el_t,
            }
        )
    return in_maps


def kernel(x, embedding):
    from concourse.bass_utils import run_bass_kernel_spmd

    x = np.ascontiguousarray(np.asarray(x, dtype=np.float32))
    e = np.ascontiguousarray(np.asarray(embedding, dtype=np.float32))
    xf = x.reshape(-1, D)
    n_total = xf.shape[0]
    assert n_total == N_CORES * TOK_PER_CORE and e.shape == (K, D)

    nc = build()
    in_maps = make_in_maps(xf, e)
    res = run_bass_kernel_spmd(nc, in_maps, list(range(N_CORES))).results

    q = np.concatenate([res[i]["q"] for i in range(N_CORES)], axis=0)
    codes = np.concatenate(
        [res[i]["codes"].reshape(-1) for i in range(N_CORES)], axis=0
    ).astype(np.int32)
    return q, xf, codes


# revision 23
# speedup vs baseline: 1.2703x; 1.0112x over previous
"""EuclideanCodebook (VQ) kernel for 8 TRN2 NeuronCores.

Problem: x [8, 4096, 256] f32, embedding [2048, 256] f32.
  xf = x.reshape(-1, 256); dist = |xf|^2 + |e|^2 - 2 xf @ e.T
  codes = argmin(dist, -1) (int32); quantized = embedding[codes]
  returns (quantized, xf, codes)

Sharding: data-parallel on tokens, 4096 per core; embedding replicated.

Numerics: the TensorEngine's fast paths are reduced precision, so the
x.e matmul uses an explicit Dekker-style split x = xh + xl, e = eh + el
(bf16 hi + bf16 residual) and computes xh.eh + xh.el + xl.eh with three
bf16 matmuls accumulated in fp32 PSUM; the -|e|^2/2 bias is applied
EXACTLY in fp32 by preloading PSUM via the scalar engine before each
accumulation group. On this problem the worst-case argmin margin
(2.8e-4) is ~30x the residual split error. argmax(x.e - |e|^2/2) ==
argmin(dist), and ties resolve to the first index, matching jnp.argmin.

The host ships the shard/embedding in transposed layout and as bf16
hi/lo pairs (the same input snapping the TensorEngine would perform on
chip); |e|^2 itself is computed on device from the f32 embedding.

Per-core device pipeline:
  stage A: DMA eh/el chunk tiles; DVE squares f32 e.T chunks + PE
  ones-matmul -> -|e_k|^2/2 row; PE broadcast matmul -> per-chunk
  [128,512] f32 bias tiles.
  main loop (32 tiles of 128 tokens):
   - DMA xh/xl tile [128d x 128t x 2 halves]
   - per 512-code chunk: ACT preloads PSUM with the f32 bias; 6 bf16
     matmuls accumulate the three split passes; ACT copies the chunk
     to SBUF scores.
   - DVE max8 + find_index8 (DVE runs ONLY these two ops back-to-back;
     interleaving any other DVE op halves their throughput)
   - gpsimd indirect-DMA gathers embedding[code] -> quantized rows
   - q rows (sync) + codes column (gpsimd) DMA out

Token <-> (partition, tile) mapping: local token n = t*128 + p.
"""

import numpy as np
from contextlib import ExitStack

P = 128
D = 256
K = 2048
N_CORES = 8
TOK_PER_CORE = 4096
NCHUNK = 4
CW = 512
KCH = 2


def build(n_tok=TOK_PER_CORE):
    import concourse.bass as bass
    import concourse.tile as tile
    from concourse import bacc, mybir
    from concourse.bass import ts

    f32 = mybir.dt.float32
    bf16 = mybir.dt.bfloat16
    u32 = mybir.dt.uint32

    TT = n_tok // P

    nc = bacc.Bacc(
        "TRN2", target_bir_lowering=False, debug=False, num_devices=N_CORES
    )

    xh_d = nc.dram_tensor("xh_t", [D, n_tok], bf16, kind="ExternalInput").ap()
    xl_d = nc.dram_tensor("xl_t", [D, n_tok], bf16, kind="ExternalInput").ap()
    e_d = nc.dram_tensor("emb", [K, D], f32, kind="ExternalInput").ap()
    et_d = nc.dram_tensor("emb_t", [D, K], f32, kind="ExternalInput").ap()
    eh_d = nc.dram_tensor("eh_t", [D, K], bf16, kind="ExternalInput").ap()
    el_d = nc.dram_tensor("el_t", [D, K], bf16, kind="ExternalInput").ap()
    q_d = nc.dram_tensor("q", [n_tok, D], f32, kind="ExternalOutput").ap()
    c_d = nc.dram_tensor("codes", [TT, P], u32, kind="ExternalOutput").ap()

    with tile.TileContext(nc) as tc, ExitStack() as ctx:
        const = ctx.enter_context(tc.tile_pool(name="const", bufs=1))
        xTpool = ctx.enter_context(tc.tile_pool(name="xTpool", bufs=4))
        spool = ctx.enter_context(tc.tile_pool(name="spool", bufs=3))
        qpool = ctx.enter_context(tc.tile_pool(name="qpool", bufs=3))
        ipool = ctx.enter_context(tc.tile_pool(name="ipool", bufs=4))
        junk = ctx.enter_context(tc.tile_pool(name="junk", bufs=2))
        epool = ctx.enter_context(tc.tile_pool(name="epool", bufs=4))
        sqpool = ctx.enter_context(tc.tile_pool(name="sqpool", bufs=4))
        psum_b2 = ctx.enter_context(tc.tile_pool(name="psum_b2", bufs=2, space="PSUM"))
        psum_mm = ctx.enter_context(tc.tile_pool(name="psum_mm", bufs=6, space="PSUM"))

        ones_col = const.tile([P, 1], f32, tag="ones_col")
        nc.vector.memset(ones_col[:], 1.0)
        ones_row = const.tile([1, P], f32, tag="ones_row")
        nc.vector.memset(ones_row[:], 1.0)

        eh = [
            [
                const.tile([P, CW], bf16, tag=f"eh{c}_{n}", name=f"eh{c}_{n}")
                for n in range(NCHUNK)
            ]
            for c in range(KCH)
        ]
        el = [
            [
                const.tile([P, CW], bf16, tag=f"el{c}_{n}", name=f"el{c}_{n}")
                for n in range(NCHUNK)
            ]
            for c in range(KCH)
        ]
        # per-chunk bias rows: a single [1, K] tile would serialize the four
        # mul -> broadcast chains through whole-tile WAR tracking
        nhb2_row = [
            const.tile([1, CW], f32, tag=f"nhb2_row{n}", name=f"nhb2_row{n}")
            for n in range(NCHUNK)
        ]
        zrow = const.tile([1, CW], f32, tag="zrow")
        nc.vector.memset(zrow[:], 0.0)
        nhb2 = [
            const.tile([P, CW], f32, tag=f"nhb2_{n}", name=f"nhb2_{n}")
            for n in range(NCHUNK)
        ]

        # Warm TWO spare PSUM slots with a start=True matmul each, first
        # thing (independent of stage A): the main loop accumulates with
        # start=False onto ACT-preloaded banks, which only adds (rather
        # than replaces) when the bank's has_written bits are already set
        # by a prior TensorEngine write.  The four stage-A broadcast
        # matmuls below warm the other four slots.
        for w in range(2):
            wb = psum_mm.tile([P, CW], f32, tag="mm", name=f"warm{w}")
            nc.tensor.matmul(
                wb[:], lhsT=ones_row[:], rhs=zrow[:], start=True, stop=True
            )
            wj = junk.tile([P, CW], f32, tag="wj", name=f"warmj{w}")
            nc.scalar.copy(wj[:], wb[:])

        # ---- stage A: load split e.T chunks, compute -|e|^2/2 ----
        # Spread stage-A loads over all three DMA queues so the sync queue
        # can start feeding the main loop's x tiles early.
        for nch in range(NCHUNK):
            for c in range(KCH):
                nc.sync.dma_start(eh[c][nch][:], eh_d[ts(c, P), ts(nch, CW)])
                nc.gpsimd.dma_start(el[c][nch][:], el_d[ts(c, P), ts(nch, CW)])
            bp = psum_b2.tile([1, CW], f32, tag="b2")
            for c in range(KCH):
                etile = epool.tile([P, CW], f32, tag="etile")
                nc.scalar.dma_start(etile[:], et_d[ts(c, P), ts(nch, CW)])
                sq = sqpool.tile([P, CW], f32, tag="sq")
                nc.vector.tensor_mul(sq[:], etile[:], etile[:])
                nc.tensor.matmul(
                    bp[:],
                    lhsT=ones_col[:],
                    rhs=sq[:],
                    start=(c == 0),
                    stop=(c == KCH - 1),
                )
            nc.scalar.mul(nhb2_row[nch][:], bp[:], -0.5)
            bb = psum_mm.tile([P, CW], f32, tag="mm")
            nc.tensor.matmul(
                bb[:],
                lhsT=ones_row[:],
                rhs=nhb2_row[nch][:],
                start=True,
                stop=True,
            )
            nc.scalar.copy(nhb2[nch][:], bb[:])

        # ---- main loop ----
        for t in range(TT):
            xh = xTpool.tile([P, D], bf16, tag="xh")
            xl = xTpool.tile([P, D], bf16, tag="xl")
            for c in range(KCH):
                nc.sync.dma_start(xh[:, ts(c, P)], xh_d[ts(c, P), ts(t, P)])
                nc.sync.dma_start(xl[:, ts(c, P)], xl_d[ts(c, P), ts(t, P)])

            scores = spool.tile([P, K], f32, tag="scores")
            for nch in range(NCHUNK):
                mm = psum_mm.tile([P, CW], f32, tag="mm")
                nc.scalar.copy(mm[:], nhb2[nch][:])  # exact f32 bias preload
                passes = [(xh, eh), (xh, el), (xl, eh)]
                for pi, (xa, ea) in enumerate(passes):
                    for c in range(KCH):
                        nc.tensor.matmul(
                            mm[:],
                            lhsT=xa[:, ts(c, P)],
                            rhs=ea[c][nch][:],
                            start=False,
                            stop=(pi == 2 and c == KCH - 1),
                            skip_group_check=True,
                        )
                nc.scalar.copy(scores[:, ts(nch, CW)], mm[:])

            v8 = ipool.tile([P, 8], f32, tag="v8")
            nc.vector.max(v8[:], scores[:])
            idx8 = ipool.tile([P, 8], u32, tag="idx8")
            nc.vector.max_index(idx8[:], v8[:], scores[:])

            qt = qpool.tile([P, D], f32, tag="qt")
            nc.gpsimd.indirect_dma_start(
                out=qt[:],
                out_offset=None,
                in_=e_d[:, :],
                in_offset=bass.IndirectOffsetOnAxis(ap=idx8[:, 0:1], axis=0),
            )
            nc.sync.dma_start(q_d[ts(t, P), :], qt[:])
            nc.gpsimd.dma_start(c_d[t : t + 1, :], idx8[:, 0:1])

    nc.compile()
    return nc


def _split_bf16(a):
    import ml_dtypes

    hi = a.astype(ml_dtypes.bfloat16)
    lo = (a - hi.astype(np.float32)).astype(ml_dtypes.bfloat16)
    return hi, lo


def make_in_maps(xf, e):
    e_t = np.ascontiguousarray(e.T)
    eh_t, el_t = _split_bf16(e_t)
    eh_t, el_t = np.ascontiguousarray(eh_t), np.ascontiguousarray(el_t)
    in_maps = []
    for i in range(N_CORES):
        shard_t = np.ascontiguousarray(
            xf[i * TOK_PER_CORE : (i + 1) * TOK_PER_CORE].T
        )
        xh_t, xl_t = _split_bf16(shard_t)
        in_maps.append(
            {
                "xh_t": np.ascontiguousarray(xh_t),
                "xl_t": np.ascontiguousarray(xl_t),
                "emb": e,
                "emb_t": e_t,
                "eh_t": eh_t,
                "el_t": el_t,
            }
        )
    return in_maps


def kernel(x, embedding):
    from concourse.bass_utils import run_bass_kernel_spmd

    x = np.ascontiguousarray(np.asarray(x, dtype=np.float32))
    e = np.ascontiguousarray(np.asarray(embedding, dtype=np.float32))
    xf = x.reshape(-1, D)
    n_total = xf.shape[0]
    assert n_total == N_CORES * TOK_PER_CORE and e.shape == (K, D)

    nc = build()
    in_maps = make_in_maps(xf, e)
    res = run_bass_kernel_spmd(nc, in_maps, list(range(N_CORES))).results

    q = np.concatenate([res[i]["q"] for i in range(N_CORES)], axis=0)
    codes = np.concatenate(
        [res[i]["codes"].reshape(-1) for i in range(N_CORES)], axis=0
    ).astype(np.int32)
    return q, xf, codes


# revision 25
# speedup vs baseline: 1.2838x; 1.0107x over previous
"""EuclideanCodebook (VQ) kernel for 8 TRN2 NeuronCores.

Problem: x [8, 4096, 256] f32, embedding [2048, 256] f32.
  xf = x.reshape(-1, 256); dist = |xf|^2 + |e|^2 - 2 xf @ e.T
  codes = argmin(dist, -1) (int32); quantized = embedding[codes]
  returns (quantized, xf, codes)

Sharding: data-parallel on tokens, 4096 per core; embedding replicated.

Numerics: the TensorEngine's fast paths are reduced precision, so the
x.e matmul uses an explicit Dekker-style split x = xh + xl, e = eh + el
(bf16 hi + bf16 residual) and computes xh.eh + xh.el + xl.eh with three
bf16 matmuls accumulated in fp32 PSUM; the -|e|^2/2 bias is applied
EXACTLY in fp32 by preloading PSUM via the scalar engine before each
accumulation group. On this problem the worst-case argmin margin
(2.8e-4) is ~30x the residual split error. argmax(x.e - |e|^2/2) ==
argmin(dist), and ties resolve to the first index, matching jnp.argmin.

The host ships the shard/embedding in transposed layout and as bf16
hi/lo pairs (the same input snapping the TensorEngine would perform on
chip); |e|^2 itself is computed on device from the f32 embedding.

Per-core device pipeline:
  stage A: DMA eh/el chunk tiles; DVE squares f32 e.T chunks + PE
  ones-matmul -> -|e_k|^2/2 row; PE broadcast matmul -> per-chunk
  [128,512] f32 bias tiles.
  main loop (32 tiles of 128 tokens):
   - DMA xh/xl tile [128d x 128t x 2 halves]
   - per 512-code chunk: ACT preloads PSUM with the f32 bias; 6 bf16
     matmuls accumulate the three split passes; ACT copies the chunk
     to SBUF scores.
   - DVE max8 + find_index8 (DVE runs ONLY these two ops back-to-back;
     interleaving any other DVE op halves their throughput)
   - gpsimd indirect-DMA gathers embedding[code] -> quantized rows
   - q rows (sync) + codes column (gpsimd) DMA out

Token <-> (partition, tile) mapping: local token n = t*128 + p.
"""

import numpy as np
from contextlib import ExitStack

P = 128
D = 256
K = 2048
N_CORES = 8
TOK_PER_CORE = 4096
NCHUNK = 4
CW = 512
KCH = 2


def build(n_tok=TOK_PER_CORE):
    import concourse.bass as bass
    import concourse.tile as tile
    from concourse import bacc, mybir
    from concourse.bass import ts

    f32 = mybir.dt.float32
    bf16 = mybir.dt.bfloat16
    u32 = mybir.dt.uint32

    TT = n_tok // P

    nc = bacc.Bacc(
        "TRN2", target_bir_lowering=False, debug=False, num_devices=N_CORES
    )

    xh_d = nc.dram_tensor("xh_t", [D, n_tok], bf16, kind="ExternalInput").ap()
    xl_d = nc.dram_tensor("xl_t", [D, n_tok], bf16, kind="ExternalInput").ap()
    e_d = nc.dram_tensor("emb", [K, D], f32, kind="ExternalInput").ap()
    et_d = nc.dram_tensor("emb_t", [D, K], f32, kind="ExternalInput").ap()
    eh_d = nc.dram_tensor("eh_t", [D, K], bf16, kind="ExternalInput").ap()
    el_d = nc.dram_tensor("el_t", [D, K], bf16, kind="ExternalInput").ap()
    nb_d = nc.dram_tensor("nhb2_bounce", [NCHUNK, CW], f32).ap()
    q_d = nc.dram_tensor("q", [n_tok, D], f32, kind="ExternalOutput").ap()
    c_d = nc.dram_tensor("codes", [TT, P], u32, kind="ExternalOutput").ap()

    with tile.TileContext(nc) as tc, ExitStack() as ctx:
        const = ctx.enter_context(tc.tile_pool(name="const", bufs=1))
        xTpool = ctx.enter_context(tc.tile_pool(name="xTpool", bufs=4))
        spool = ctx.enter_context(tc.tile_pool(name="spool", bufs=3))
        qpool = ctx.enter_context(tc.tile_pool(name="qpool", bufs=3))
        ipool = ctx.enter_context(tc.tile_pool(name="ipool", bufs=4))
        junk = ctx.enter_context(tc.tile_pool(name="junk", bufs=2))
        epool = ctx.enter_context(tc.tile_pool(name="epool", bufs=4))
        sqpool = ctx.enter_context(tc.tile_pool(name="sqpool", bufs=4))
        psum_b2 = ctx.enter_context(tc.tile_pool(name="psum_b2", bufs=2, space="PSUM"))
        psum_mm = ctx.enter_context(tc.tile_pool(name="psum_mm", bufs=6, space="PSUM"))

        ones_col = const.tile([P, 1], f32, tag="ones_col")
        nc.vector.memset(ones_col[:], 1.0)
        ones_row = const.tile([1, P], f32, tag="ones_row")
        nc.vector.memset(ones_row[:], 1.0)

        eh = [
            [
                const.tile([P, CW], bf16, tag=f"eh{c}_{n}", name=f"eh{c}_{n}")
                for n in range(NCHUNK)
            ]
            for c in range(KCH)
        ]
        el = [
            [
                const.tile([P, CW], bf16, tag=f"el{c}_{n}", name=f"el{c}_{n}")
                for n in range(NCHUNK)
            ]
            for c in range(KCH)
        ]
        # per-chunk bias rows: a single [1, K] tile would serialize the four
        # mul -> broadcast chains through whole-tile WAR tracking
        nhb2_row = [
            const.tile([1, CW], f32, tag=f"nhb2_row{n}", name=f"nhb2_row{n}")
            for n in range(NCHUNK)
        ]
        zrow = const.tile([1, CW], bf16, tag="zrow")
        nc.vector.memset(zrow[:], 0.0)
        ones_row_b = const.tile([1, P], bf16, tag="ones_row_b")
        nc.vector.memset(ones_row_b[:], 1.0)
        nhb2 = [
            const.tile([P, CW], f32, tag=f"nhb2_{n}", name=f"nhb2_{n}")
            for n in range(NCHUNK)
        ]

        # Warm TWO spare PSUM slots with a start=True matmul each, first
        # thing (independent of stage A): the main loop accumulates with
        # start=False onto ACT-preloaded banks, which only adds (rather
        # than replaces) when the bank's has_written bits are already set
        # by a prior TensorEngine write.  The four stage-A broadcast
        # matmuls below warm the other four slots.
        wj = junk.tile([P, 8], f32, tag="wj")
        for w in range(6):
            wb = psum_mm.tile([P, CW], f32, tag="mm", name=f"warm{w}")
            nc.tensor.matmul(
                wb[:], lhsT=ones_row_b[:], rhs=zrow[:], start=True, stop=True
            )
            nc.scalar.copy(wj[:, w : w + 1], wb[:, 0:1])

        # ---- stage A: load split e.T chunks, compute -|e|^2/2 ----
        # Spread stage-A loads over all three DMA queues so the sync queue
        # can start feeding the main loop's x tiles early.
        for nch in range(NCHUNK):
            for c in range(KCH):
                nc.sync.dma_start(eh[c][nch][:], eh_d[ts(c, P), ts(nch, CW)])
                nc.gpsimd.dma_start(el[c][nch][:], el_d[ts(c, P), ts(nch, CW)])
            bp = psum_b2.tile([1, CW], f32, tag="b2")
            for c in range(KCH):
                etile = epool.tile([P, CW], f32, tag="etile")
                nc.scalar.dma_start(etile[:], et_d[ts(c, P), ts(nch, CW)])
                sq = sqpool.tile([P, CW], f32, tag="sq")
                if (2 * nch + c) % 2 == 0:
                    nc.vector.tensor_mul(sq[:], etile[:], etile[:])
                else:
                    nc.scalar.activation(
                        sq[:], etile[:], mybir.ActivationFunctionType.Square
                    )
                nc.tensor.matmul(
                    bp[:],
                    lhsT=ones_col[:],
                    rhs=sq[:],
                    start=(c == 0),
                    stop=(c == KCH - 1),
                )
            nc.scalar.mul(nhb2_row[nch][:], bp[:], -0.5)
            # replicate the row across partitions with a stride-0 DMA via a
            # DRAM bounce (keeps the PE/ACT queues out of the startup chain)
            nc.scalar.dma_start(nb_d[nch : nch + 1, :], nhb2_row[nch][:])
            nc.scalar.dma_start(
                nhb2[nch][:], nb_d[nch : nch + 1, :].to_broadcast([P, CW])
            )

        # ---- main loop ----
        for t in range(TT):
            xh = xTpool.tile([P, D], bf16, tag="xh")
            xl = xTpool.tile([P, D], bf16, tag="xl")
            for c in range(KCH):
                nc.sync.dma_start(xh[:, ts(c, P)], xh_d[ts(c, P), ts(t, P)])
                nc.sync.dma_start(xl[:, ts(c, P)], xl_d[ts(c, P), ts(t, P)])

            scores = spool.tile([P, K], f32, tag="scores")
            for nch in range(NCHUNK):
                mm = psum_mm.tile([P, CW], f32, tag="mm")
                nc.scalar.copy(mm[:], nhb2[nch][:])  # exact f32 bias preload
                passes = [(xh, eh), (xh, el), (xl, eh)]
                for pi, (xa, ea) in enumerate(passes):
                    for c in range(KCH):
                        nc.tensor.matmul(
                            mm[:],
                            lhsT=xa[:, ts(c, P)],
                            rhs=ea[c][nch][:],
                            start=False,
                            stop=(pi == 2 and c == KCH - 1),
                            skip_group_check=True,
                        )
                nc.scalar.copy(scores[:, ts(nch, CW)], mm[:])

            v8 = ipool.tile([P, 8], f32, tag="v8")
            nc.vector.max(v8[:], scores[:])
            idx8 = ipool.tile([P, 8], u32, tag="idx8")
            nc.vector.max_index(idx8[:], v8[:], scores[:])

            qt = qpool.tile([P, D], f32, tag="qt")
            nc.gpsimd.indirect_dma_start(
                out=qt[:],
                out_offset=None,
                in_=e_d[:, :],
                in_offset=bass.IndirectOffsetOnAxis(ap=idx8[:, 0:1], axis=0),
            )
            nc.sync.dma_start(q_d[ts(t, P), :], qt[:])
            nc.gpsimd.dma_start(c_d[t : t + 1, :], idx8[:, 0:1])

    nc.compile()
    return nc


def _split_bf16(a):
    import ml_dtypes

    hi = a.astype(ml_dtypes.bfloat16)
    lo = (a - hi.astype(np.float32)).astype(ml_dtypes.bfloat16)
    return hi, lo


def make_in_maps(xf, e):
    e_t = np.ascontiguousarray(e.T)
    eh_t, el_t = _split_bf16(e_t)
    eh_t, el_t = np.ascontiguousarray(eh_t), np.ascontiguousarray(el_t)
    in_maps = []
    for i in range(N_CORES):
        shard_t = np.ascontiguousarray(
            xf[i * TOK_PER_CORE : (i + 1) * TOK_PER_CORE].T
        )
        xh_t, xl_t = _split_bf16(shard_t)
        in_maps.append(
            {
                "xh_t": np.ascontiguousarray(xh_t),
                "xl_t": np.ascontiguousarray(xl_t),
                "emb": e,
                "emb_t": e_t,
                "eh_t": eh_t,
                "el_t": el_t,
            }
        )
    return in_maps


def kernel(x, embedding):
    from concourse.bass_utils import run_bass_kernel_spmd

    x = np.ascontiguousarray(np.asarray(x, dtype=np.float32))
    e = np.ascontiguousarray(np.asarray(embedding, dtype=np.float32))
    xf = x.reshape(-1, D)
    n_total = xf.shape[0]
    assert n_total == N_CORES * TOK_PER_CORE and e.shape == (K, D)

    nc = build()
    in_maps = make_in_maps(xf, e)
    res = run_bass_kernel_spmd(nc, in_maps, list(range(N_CORES))).results

    q = np.concatenate([res[i]["q"] for i in range(N_CORES)], axis=0)
    codes = np.concatenate(
        [res[i]["codes"].reshape(-1) for i in range(N_CORES)], axis=0
    ).astype(np.int32)
    return q, xf, codes
